# revision 55
# baseline (speedup 1.0000x reference)
"""Trainium2 Bass kernel for nn_EntropyConv (masked 5x5 PixelCNN-style conv,
per-latitude-partition padding + width masking + PReLU).

Strategy: data-parallel over batch (8 cores x 1 batch element). Per core,
a row-phase-split SBUF layout puts (row mod 4, ci) on the 128 K-partitions
so each PSUM tile computes 4 output rows x 32 channels with K=M=128 fp32r
matmuls: 10 matmuls per tile (5 kw shifts x 2 row-windows), kh taps encoded
in host-precomputed block-Toeplitz weight matrices.
"""

import sys
import os
from contextlib import ExitStack

import numpy as np

sys.path.insert(0, "/opt/trn_rl_repo")

import concourse.bass as bass  # noqa: E402
import concourse.tile as tile  # noqa: E402
from concourse import bacc, mybir  # noqa: E402
from concourse import bass_utils  # noqa: E402
from concourse.bass_utils import run_bass_kernel_spmd  # noqa: E402

# Enable walrus's redundant-LDWEIGHTS elimination: our matmul stream reuses
# each stationary weight across 8 consecutive matmuls, and the default
# --enable-ldw-opt=false forces a ~190ns weight reload per matmul (~40% of
# PE time). Correctness is verified against the fp32 reference.
if not os.environ.get("BASS_NO_LDWOPT"):
    _orig_run_command = bass_utils.run_command

    def _run_command_ldwopt(argv, **kwargs):
        argv = ["--enable-ldw-opt=true" if a == "--enable-ldw-opt=false" else a
                for a in argv]
        return _orig_run_command(argv, **kwargs)

    if bass_utils.run_command is not _run_command_ldwopt:
        bass_utils.run_command = _run_command_ldwopt

# Model constants (hardcoded per problem spec)
NGROUPS, CIN, COUT, KSIZE, NPART = 8, 4, 4, 5, 8
B, H, W = 8, 256, 512
CI = NGROUPS * CIN   # 32
CO = NGROUPS * COUT  # 32
Hp = H // NPART      # 32 rows per latitude chunk
NBLK = Hp // 4       # 8 four-row blocks per chunk
NCORES = 8
F32 = mybir.dt.float32
F32R = mybir.dt.float32r
BF16 = mybir.dt.bfloat16

# X4 tile: 2 guard cols + 9 blocks at pitch (width+4); the 4-col gap after
# each block's data doubles as its right zero-strip and the next block's
# left guard.  Allocated for the widest chunk (502).
XBLK = 9
WMAX = 504  # widths are even and <= 502
XLEN = 2 + XBLK * (WMAX + 4)

LAST_RESULT = None  # BassKernelResults from the most recent run (for test.py)


def _group_mask():
    """PixelCNN group mask for 5x5 kernel, mask-B (hidden) variant."""
    m = np.zeros((CO, CI, KSIZE, KSIZE), np.float32)
    c = KSIZE // 2
    m[:, :, :c, :] = 1.0
    m[:, :, c, :c] = 1.0
    gin = np.arange(CI) // CIN
    gout = np.arange(CO) // COUT
    center = gin[None, :] <= gout[:, None]
    m[:, :, c, c] = center.astype(np.float32)
    return m


def _build_weights(weight):
    """Block-Toeplitz lhsT matrices.

    w1/w2[kw, 32*rp+ci, 32*j+co]: contribution of input row (4h4+rp-2)
    [w1] or (4h4+rp+2) [w2] to output row (4h4+j), i.e. kh = rp-j [w1]
    or rp-j+4 [w2], valid when 0 <= kh < 5.
    """
    wm = (weight * _group_mask()).astype(np.float32)  # [co, ci, kh, kw]
    w1 = np.zeros((KSIZE, 128, 128), np.float32)
    w2 = np.zeros((KSIZE, 128, 128), np.float32)
    for rp in range(4):
        for j in range(4):
            kh1 = rp - j
            kh2 = rp - j + 4
            # [ci, co] block at rows 32*rp+ci, cols 4*co+j (co-major
            # output partitions -> contiguous 4-row HBM stores)
            if 0 <= kh1 < KSIZE:
                for kw in range(KSIZE):
                    w1[kw, 32 * rp:32 * rp + 32, j::4] = wm[:, :, kh1, kw].T
            if 0 <= kh2 < KSIZE:
                for kw in range(KSIZE):
                    w2[kw, 32 * rp:32 * rp + 32, j::4] = wm[:, :, kh2, kw].T
    return w1, w2


def _tile_groups(width):
    """Split the 8 output blocks of a chunk into PSUM tile groups of k
    blocks, keeping k*width <= 512 (one PSUM bank) and preferring
    k*width >= 256 (fp32r full-rate threshold)."""
    if width >= 256:
        return [(b, 1) for b in range(NBLK)]
    k = min(NBLK, 512 // width)
    groups = []
    b = 0
    while b < NBLK:
        kk = min(k, NBLK - b)
        groups.append((b, kk))
        b += kk
    return groups


def _build_program(widths, has_bias, use_prelu=True):
    nc = bacc.Bacc("TRN2", target_bir_lowering=False, debug=False,
                   num_devices=NCORES)

    x_d = nc.dram_tensor("x", [CI, H, W], F32R, kind="ExternalInput")
    w1_d = nc.dram_tensor("w1", [KSIZE, 128, 128], F32R, kind="ExternalInput")
    w2_d = nc.dram_tensor("w2", [KSIZE, 128, 128], F32R, kind="ExternalInput")
    alpha_d = nc.dram_tensor("alpha_p", [128, 1], F32, kind="ExternalInput")
    if has_bias:
        bias_d = nc.dram_tensor("bias_p", [128, 1], F32, kind="ExternalInput")
    y_d = nc.dram_tensor("y", [CO, H, W], F32, kind="ExternalOutput")

    # DRAM views for phase-split access
    # x rows: global row = 4*hb + r
    x_r = x_d.ap().rearrange("ci (hb r) w -> r ci hb w", r=4)

    with tile.TileContext(nc) as tc, ExitStack() as ctx:
        wpool = ctx.enter_context(tc.tile_pool(name="wts", bufs=1))
        spool = ctx.enter_context(tc.tile_pool(name="scalars", bufs=1))
        x4pool = ctx.enter_context(tc.tile_pool(name="x4", bufs=1))
        psumpool = ctx.enter_context(
            tc.tile_pool(name="psum", bufs=8, space=bass.MemorySpace.PSUM))
        outpool = ctx.enter_context(tc.tile_pool(name="outsb", bufs=8))

        # Persistent x4 buffers (manual rotation p -> p%4).
        NX4 = 4
        x4bufs = [x4pool.tile([128, XLEN], F32R, tag=f"x4_{i}",
                              name=f"x4_{i}")
                  for i in range(NX4)]

        wt1 = wpool.tile([128, KSIZE, 128], F32R, tag="w1")
        wt2 = wpool.tile([128, KSIZE, 128], F32R, tag="w2")
        w1v = w1_d.ap().rearrange("kw k m -> k kw m")
        w2v = w2_d.ap().rearrange("kw k m -> k kw m")
        alpha_t = spool.tile([128, 1], F32, tag="alpha")
        if has_bias:
            bias_t = spool.tile([128, 1], F32, tag="bias")
        # wt2/alpha ride the scalar queue (idle until the first PRELU);
        # wt1 is issued on sync after chunk 0's assist loads (below) since
        # chunk-0 data is the critical path to the first matmul

        prev_mm = [None]
        st_rr = [0]

        # fused-load views: rows 4b+r for r in 0..3 -> [r, ci, hb, w]
        x_r4 = x_d.ap().rearrange("ci (hb r) w -> r ci hb w", r=4)

        seen_bufs = set()

        def issue_chunk_inputs(q, buf, assist=False):
            """Queue chunk q's memsets + input loads (no compute deps)."""
            width = widths[q]
            PW = width + 4
            x4f = x4bufs[buf][:, :]
            bv = x4f[:, 2:2 + XBLK * PW].rearrange("q (b x) -> q b x", x=PW)
            if buf not in seen_bufs:
                seen_bufs.add(buf)
                # head guard: never DMA-written, zero once per buffer
                nc.vector.memset(x4f[:, 0:2].bitcast(F32), 0.0)
            # per-chunk zeroing: 4-col gap strips (right strip of block b +
            # left guard of block b+1) and the two half-height pad blocks
            nc.vector.memset(bv[:, :, width:width + 4].bitcast(F32), 0.0)
            nc.vector.memset(bv[0:64, 0, 0:width].bitcast(F32), 0.0)
            # 4 rp loads (3D APs: DMA supports at most 3 dims per side)
            for rp in range(4):
                r = rp + 2 if rp < 2 else rp - 2
                bdst = 1 if rp < 2 else 0
                src = x_r4[r][:, q * NBLK:q * NBLK + NBLK, 0:width]
                dst = x4f[32 * rp:32 * rp + 32,
                          2 + bdst * PW:2 + (bdst + NBLK) * PW].rearrange(
                              "q (b x) -> q b x", x=PW)[:, :, 0:width]
                if assist:
                    eng = (nc.gpsimd, nc.sync)[rp % 2]
                else:
                    eng = nc.gpsimd
                eng.dma_start(dst, src)
            nc.vector.memset(bv[64:128, 8, 0:width].bitcast(F32), 0.0)

        # Natural order: chunk 0 (width 98) loads fastest -> earliest PE
        # start; chunks 6,7 (284, 98) keep the tail store flush small.
        ORDER = list(range(NPART))

        # software pipeline: keep PREFETCH chunks' loads queued ahead of the
        # compute stream so no store/PRELU dependency can block a prefetch
        # Buffers are assigned by PROCESSING position (pi % NX4): the issue
        # of position pi+PREFETCH targets buffer (pi-1)%NX4, whose previous
        # tenant (position pi-1) has already been fully emitted — keeping
        # program order consistent with the WAR rotation.
        PREFETCH = 3
        # Dummy-matmul scratch: a region of buffer 0 no early chunk's DMA
        # touches (chunk 0 at pitch 102 uses cols < 920).  Dummy matmuls
        # keep the PE busy through the load-bound head stalls so the clock
        # ramp (0.65 -> 1.2 -> 2.4GHz after 3us continuous) is warm when
        # real data lands; results go to a PSUM tile that the next real
        # group's start=True matmul resets, so nothing is polluted.
        dscr = x4bufs[0][:, 1024:1536]
        nc.vector.memset(dscr.bitcast(F32), 0.0)
        dmy_w = x4bufs[0][:, 1024:1152]
        dmy_ps = psumpool.tile([128, 512], F32, tag="ps")

        def emit_dummies(n, rhs_ap):
            for _ in range(n):
                mm = nc.tensor.matmul(dmy_ps[:, :], dmy_w, rhs_ap,
                                      start=True, stop=True)
                if prev_mm[0] is not None:
                    bass._add_dep_helper(
                        mm.ins, prev_mm[0].ins, sync=False,
                        reason="pe-stream-order")
                prev_mm[0] = mm

        nc.sync.dma_start(wt1[:, :, :], w1v)
        issue_chunk_inputs(ORDER[0], 0, assist=True)
        nc.scalar.dma_start(wt2[:, :, :], w2v)
        nc.scalar.dma_start(alpha_t[:], alpha_d.ap())
        if has_bias:
            nc.scalar.dma_start(bias_t[:], bias_d.ap())
        for i in range(1, PREFETCH):
            issue_chunk_inputs(ORDER[i], i % NX4, assist=True)

        emit_dummies(8, dscr)  # pre-warm while chunk-0 loads are in flight

        for pi in range(NPART):
            p = ORDER[pi]
            width = widths[p]
            PW = width + 4
            x4 = x4bufs[pi % NX4]
            x4f = x4[:, :]

            if pi + PREFETCH < NPART:
                issue_chunk_inputs(ORDER[pi + PREFETCH],
                                   (pi + PREFETCH) % NX4)

            all_groups = _tile_groups(width)
            if pi >= NPART - 2:
                # tail chunks: tile-major so postproc drains immediately
                halves = [[g] for g in all_groups]
            elif len(all_groups) >= 6:
                halves = [all_groups[0:4], all_groups[4:8]]
            else:
                halves = [all_groups[:(len(all_groups) + 1) // 2],
                          all_groups[(len(all_groups) + 1) // 2:]]

            for groups in halves:
              if not groups:
                  continue
              psums = []
              for (b0, k) in groups:
                ps_t = psumpool.tile([128, k * width], F32, tag="ps")
                psums.append(ps_t)

              # weight-major: each stationary weight is reused across all
              # groups back-to-back so walrus's ldw-opt elides the reloads
              NW = 2 * KSIZE
              for wi in range(NW):
                m, kw = divmod(wi, KSIZE)
                lhsT = (wt1 if m == 0 else wt2)[:, kw, :]
                for gi, (b0, k) in enumerate(groups):
                    s = 2 + (b0 + m) * PW + (kw - 2)
                    rhs = x4f[:, s:s + k * PW].rearrange(
                        "q (b x) -> q b x", x=PW)[:, :, 0:width]
                    pview = psums[gi][:, :].rearrange(
                        "q (b x) -> q b x", x=width)
                    mm = nc.tensor.matmul(
                        pview,
                        lhsT,
                        rhs,
                        start=(wi == 0),
                        stop=(wi == NW - 1),
                    )
                    if prev_mm[0] is not None:
                        bass._add_dep_helper(
                            mm.ins, prev_mm[0].ins, sync=False,
                            reason="pe-stream-order")
                    prev_mm[0] = mm

              # PRELU each group into a shared per-half SBUF tile, then one
              # fused store for the half (DMA descriptors cost ~620ns of
              # queue time each, so fewer+bigger transfers win)
              G = sum(k for (_, k) in groups)
              hb0 = p * NBLK + groups[0][0]
              out_sb = outpool.tile([128, G * width], F32, tag="osb")
              off = 0
              for gi, (b0, k) in enumerate(groups):
                n = k * width
                # single ACT op: out = prelu(psum + bias, alpha)
                # (HW-verified exact; CoreSim lacks Prelu)
                nc.scalar.activation(
                    out_sb[:, off:off + n], psums[gi][:, :],
                    mybir.ActivationFunctionType.Prelu,
                    bias=(bias_t[:, :] if has_bias else 0.0),
                    scale=1.0, alpha=alpha_t[:, :])
                off += n
              # Per-block stores (a fused multi-block store needs a 4-dim
              # AP, which the DMA engine rejects).  All on sync: a store on
              # gpsimd would block later chunks' prefetch loads behind its
              # PRELU dependency (in-order queue).  Near the end gpsimd and
              # scalar have no further work, so spread for faster drain.
              off = 0
              for gi, (b0, k) in enumerate(groups):
                  for bb in range(k):
                      hb = p * NBLK + b0 + bb
                      dst = y_d.ap()[:, 4 * hb:4 * hb + 4, 0:width]
                      if pi >= NPART - 4:
                          # spread stores over sync+scalar (~110GB/s each)
                          # so no backlog remains at the end.  gpsimd never
                          # stores: its software DMA ring has a multi-us
                          # drain-poll latency that would gate kernel end.
                          engs = (nc.sync, nc.scalar)
                      else:
                          engs = (nc.sync,)
                      eng = engs[st_rr[0] % len(engs)]
                      st_rr[0] += 1
                      eng.dma_start(dst, out_sb[:, off:off + width])
                      off += width

            if pi < 3:
                # fill the load-bound inter-chunk stall with dummy matmuls
                # on this chunk's (already valid) data so the clock ramp
                # stays warm while the next chunk's transfers finish
                emit_dummies((6, 4, 3)[pi], x4f[:, 2:514])

    nc.compile()
    return nc


def kernel(x, weight, bias, alpha, widths, _trace=False):
    global LAST_RESULT
    x = np.ascontiguousarray(np.asarray(x, dtype=np.float32))
    weight = np.asarray(weight, dtype=np.float32)
    bias = np.asarray(bias, dtype=np.float32)
    alpha = np.asarray(alpha, dtype=np.float32)
    widths_np = np.asarray(widths, dtype=np.int32)
    wlist = [int(v) for v in widths_np]
    assert x.shape == (B, CI, H, W)
    for wv in wlist:
        # the block-wraparound trick requires masked-zero cols at [510,512)
        assert 4 <= wv <= W - 6, f"width {wv} outside supported range"

    w1, w2 = _build_weights(weight)
    alpha_p = np.ascontiguousarray(np.repeat(alpha, 4)[:, None].astype(np.float32))
    has_bias = bool(np.any(bias != 0.0))

    nc = _build_program(wlist, has_bias)

    shared = {"w1": w1, "w2": w2, "alpha_p": alpha_p}
    if has_bias:
        shared["bias_p"] = np.ascontiguousarray(
            np.repeat(bias, 4)[:, None].astype(np.float32))
    in_maps = [dict(shared, x=np.ascontiguousarray(x[b])) for b in range(B)]

    res = run_bass_kernel_spmd(nc, in_maps, list(range(NCORES)),
                               trace=_trace)
    LAST_RESULT = res
    y = np.stack([res.results[c]["y"] for c in range(NCORES)], axis=0)
    return y.astype(np.float32)


if __name__ == "__main__":
    # smoke test with random data (no reference comparison)
    rng = np.random.default_rng(0)
    x = rng.standard_normal((B, CI, H, W), dtype=np.float32)
    weight = (rng.standard_normal((CO, CI, 5, 5)) * 0.05).astype(np.float32)
    bias = np.zeros(CO, np.float32)
    alpha = np.full(CO, 0.25, np.float32)
    lat = (np.arange(NPART) + 0.5) / NPART * np.pi - np.pi / 2.0
    widths = np.maximum(((np.cos(lat) * W).astype(np.int32) // 2) * 2, 16)
    y = kernel(x, weight, bias, alpha, widths.astype(np.int32))
    print("out", y.shape, y.dtype, float(np.abs(y).max()))



# revision 56
# speedup vs baseline: 1.1504x; 1.1504x over previous
"""Trainium2 Bass kernel for nn_EntropyConv (masked 5x5 PixelCNN-style conv,
per-latitude-partition padding + width masking + PReLU).

Strategy: data-parallel over batch (8 cores x 1 batch element). Per core,
a row-phase-split SBUF layout puts (row mod 4, ci) on the 128 K-partitions
so each PSUM tile computes 4 output rows x 32 channels with K=M=128 fp32r
matmuls: 10 matmuls per tile (5 kw shifts x 2 row-windows), kh taps encoded
in host-precomputed block-Toeplitz weight matrices.
"""

import sys
import os
from contextlib import ExitStack

import numpy as np

sys.path.insert(0, "/opt/trn_rl_repo")

import concourse.bass as bass  # noqa: E402
import concourse.tile as tile  # noqa: E402
from concourse import bacc, mybir  # noqa: E402
from concourse import bass_utils  # noqa: E402
from concourse.bass_utils import run_bass_kernel_spmd  # noqa: E402

# Enable walrus's redundant-LDWEIGHTS elimination: our matmul stream reuses
# each stationary weight across 8 consecutive matmuls, and the default
# --enable-ldw-opt=false forces a ~190ns weight reload per matmul (~40% of
# PE time). Correctness is verified against the fp32 reference.
if not os.environ.get("BASS_NO_LDWOPT"):
    _orig_run_command = bass_utils.run_command

    def _run_command_ldwopt(argv, **kwargs):
        argv = ["--enable-ldw-opt=true" if a == "--enable-ldw-opt=false" else a
                for a in argv]
        return _orig_run_command(argv, **kwargs)

    if bass_utils.run_command is not _run_command_ldwopt:
        bass_utils.run_command = _run_command_ldwopt

# Model constants (hardcoded per problem spec)
NGROUPS, CIN, COUT, KSIZE, NPART = 8, 4, 4, 5, 8
B, H, W = 8, 256, 512
CI = NGROUPS * CIN   # 32
CO = NGROUPS * COUT  # 32
Hp = H // NPART      # 32 rows per latitude chunk
NBLK = Hp // 4       # 8 four-row blocks per chunk
NCORES = 8
F32 = mybir.dt.float32
F32R = mybir.dt.float32r
BF16 = mybir.dt.bfloat16

# X4 tile: 2 guard cols + 9 blocks at pitch (width+4); the 4-col gap after
# each block's data doubles as its right zero-strip and the next block's
# left guard.  Allocated for the widest chunk (502).
XBLK = 9
WMAX = 504  # widths are even and <= 502
XLEN = 2 + XBLK * (WMAX + 4)

LAST_RESULT = None  # BassKernelResults from the most recent run (for test.py)


def _group_mask():
    """PixelCNN group mask for 5x5 kernel, mask-B (hidden) variant."""
    m = np.zeros((CO, CI, KSIZE, KSIZE), np.float32)
    c = KSIZE // 2
    m[:, :, :c, :] = 1.0
    m[:, :, c, :c] = 1.0
    gin = np.arange(CI) // CIN
    gout = np.arange(CO) // COUT
    center = gin[None, :] <= gout[:, None]
    m[:, :, c, c] = center.astype(np.float32)
    return m


def _build_weights(weight):
    """Block-Toeplitz lhsT matrices.

    w1/w2[kw, 32*rp+ci, 32*j+co]: contribution of input row (4h4+rp-2)
    [w1] or (4h4+rp+2) [w2] to output row (4h4+j), i.e. kh = rp-j [w1]
    or rp-j+4 [w2], valid when 0 <= kh < 5.
    """
    wm = (weight * _group_mask()).astype(np.float32)  # [co, ci, kh, kw]
    w1 = np.zeros((KSIZE, 128, 128), np.float32)
    w2 = np.zeros((KSIZE, 128, 128), np.float32)
    for rp in range(4):
        for j in range(4):
            kh1 = rp - j
            kh2 = rp - j + 4
            # [ci, co] block at rows 32*rp+ci, cols 4*co+j (co-major
            # output partitions -> contiguous 4-row HBM stores)
            if 0 <= kh1 < KSIZE:
                for kw in range(KSIZE):
                    w1[kw, 32 * rp:32 * rp + 32, j::4] = wm[:, :, kh1, kw].T
            if 0 <= kh2 < KSIZE:
                for kw in range(KSIZE):
                    w2[kw, 32 * rp:32 * rp + 32, j::4] = wm[:, :, kh2, kw].T
    return w1, w2


def _tile_groups(width):
    """Split the 8 output blocks of a chunk into PSUM tile groups of k
    blocks, keeping k*width <= 512 (one PSUM bank) and preferring
    k*width >= 256 (fp32r full-rate threshold)."""
    if width >= 256:
        return [(b, 1) for b in range(NBLK)]
    k = min(NBLK, 512 // width)
    groups = []
    b = 0
    while b < NBLK:
        kk = min(k, NBLK - b)
        groups.append((b, kk))
        b += kk
    return groups


def _build_program(widths, has_bias, use_prelu=True):
    nc = bacc.Bacc("TRN2", target_bir_lowering=False, debug=False,
                   num_devices=NCORES)

    x_d = nc.dram_tensor("x", [CI, H, W], F32R, kind="ExternalInput")
    w1_d = nc.dram_tensor("w1", [KSIZE, 128, 128], F32R, kind="ExternalInput")
    w2_d = nc.dram_tensor("w2", [KSIZE, 128, 128], F32R, kind="ExternalInput")
    alpha_d = nc.dram_tensor("alpha_p", [128, 1], F32, kind="ExternalInput")
    if has_bias:
        bias_d = nc.dram_tensor("bias_p", [128, 1], F32, kind="ExternalInput")
    y_d = nc.dram_tensor("y", [CO, H, W], F32, kind="ExternalOutput")

    # DRAM views for phase-split access
    # x rows: global row = 4*hb + r
    x_r = x_d.ap().rearrange("ci (hb r) w -> r ci hb w", r=4)

    with tile.TileContext(nc) as tc, ExitStack() as ctx:
        wpool = ctx.enter_context(tc.tile_pool(name="wts", bufs=1))
        spool = ctx.enter_context(tc.tile_pool(name="scalars", bufs=1))
        x4pool = ctx.enter_context(tc.tile_pool(name="x4", bufs=1))
        psumpool = ctx.enter_context(
            tc.tile_pool(name="psum", bufs=8, space=bass.MemorySpace.PSUM))
        outpool = ctx.enter_context(tc.tile_pool(name="outsb", bufs=8))

        # Persistent x4 buffers (manual rotation p -> p%4).
        NX4 = 4
        x4bufs = [x4pool.tile([128, XLEN], F32R, tag=f"x4_{i}",
                              name=f"x4_{i}")
                  for i in range(NX4)]

        wt1 = wpool.tile([128, KSIZE, 128], F32R, tag="w1")
        wt2 = wpool.tile([128, KSIZE, 128], F32R, tag="w2")
        w1v = w1_d.ap().rearrange("kw k m -> k kw m")
        w2v = w2_d.ap().rearrange("kw k m -> k kw m")
        alpha_t = spool.tile([128, 1], F32, tag="alpha")
        if has_bias:
            bias_t = spool.tile([128, 1], F32, tag="bias")
        # wt2/alpha ride the scalar queue (idle until the first PRELU);
        # wt1 is issued on sync after chunk 0's assist loads (below) since
        # chunk-0 data is the critical path to the first matmul

        prev_mm = [None]
        st_rr = [0]

        # fused-load views: rows 4b+r for r in 0..3 -> [r, ci, hb, w]
        x_r4 = x_d.ap().rearrange("ci (hb r) w -> r ci hb w", r=4)

        seen_bufs = set()

        def issue_chunk_inputs(q, buf, assist=False):
            """Queue chunk q's memsets + input loads (no compute deps)."""
            width = widths[q]
            PW = width + 4
            x4f = x4bufs[buf][:, :]
            bv = x4f[:, 2:2 + XBLK * PW].rearrange("q (b x) -> q b x", x=PW)
            if buf not in seen_bufs:
                seen_bufs.add(buf)
                # head guard: never DMA-written, zero once per buffer
                nc.vector.memset(x4f[:, 0:2].bitcast(F32), 0.0)
            # per-chunk zeroing: 4-col gap strips (right strip of block b +
            # left guard of block b+1) and the two half-height pad blocks
            nc.vector.memset(bv[:, :, width:width + 4].bitcast(F32), 0.0)
            nc.vector.memset(bv[0:64, 0, 0:width].bitcast(F32), 0.0)
            # 4 rp loads (3D APs: DMA supports at most 3 dims per side)
            for rp in range(4):
                r = rp + 2 if rp < 2 else rp - 2
                bdst = 1 if rp < 2 else 0
                src = x_r4[r][:, q * NBLK:q * NBLK + NBLK, 0:width]
                dst = x4f[32 * rp:32 * rp + 32,
                          2 + bdst * PW:2 + (bdst + NBLK) * PW].rearrange(
                              "q (b x) -> q b x", x=PW)[:, :, 0:width]
                if assist:
                    eng = (nc.gpsimd, nc.sync)[rp % 2]
                else:
                    eng = nc.gpsimd
                eng.dma_start(dst, src)
            nc.vector.memset(bv[64:128, 8, 0:width].bitcast(F32), 0.0)

        # Natural order: chunk 0 (width 98) loads fastest -> earliest PE
        # start; chunks 6,7 (284, 98) keep the tail store flush small.
        ORDER = list(range(NPART))

        # software pipeline: keep PREFETCH chunks' loads queued ahead of the
        # compute stream so no store/PRELU dependency can block a prefetch
        # Buffers are assigned by PROCESSING position (pi % NX4): the issue
        # of position pi+PREFETCH targets buffer (pi-1)%NX4, whose previous
        # tenant (position pi-1) has already been fully emitted — keeping
        # program order consistent with the WAR rotation.
        PREFETCH = 3
        # Dummy-matmul scratch: a region of buffer 0 no early chunk's DMA
        # touches (chunk 0 at pitch 102 uses cols < 920).  Dummy matmuls
        # keep the PE busy through the load-bound head stalls so the clock
        # ramp (0.65 -> 1.2 -> 2.4GHz after 3us continuous) is warm when
        # real data lands; results go to a PSUM tile that the next real
        # group's start=True matmul resets, so nothing is polluted.
        dscr = x4bufs[0][:, 1024:1536]
        nc.vector.memset(dscr.bitcast(F32), 0.0)
        dmy_w = x4bufs[0][:, 1024:1152]
        dmy_ps = psumpool.tile([128, 512], F32, tag="ps")

        def emit_dummies(n, rhs_ap):
            for _ in range(n):
                mm = nc.tensor.matmul(dmy_ps[:, :], dmy_w, rhs_ap,
                                      start=True, stop=True)
                if prev_mm[0] is not None:
                    bass._add_dep_helper(
                        mm.ins, prev_mm[0].ins, sync=False,
                        reason="pe-stream-order")
                prev_mm[0] = mm

        nc.sync.dma_start(wt1[:, :, :], w1v)
        issue_chunk_inputs(ORDER[0], 0, assist=True)
        nc.scalar.dma_start(wt2[:, :, :], w2v)
        nc.scalar.dma_start(alpha_t[:], alpha_d.ap())
        if has_bias:
            nc.scalar.dma_start(bias_t[:], bias_d.ap())
        for i in range(1, PREFETCH):
            issue_chunk_inputs(ORDER[i], i % NX4, assist=True)

        emit_dummies(8, dscr)  # pre-warm while chunk-0 loads are in flight

        for pi in range(NPART):
            p = ORDER[pi]
            width = widths[p]
            PW = width + 4
            x4 = x4bufs[pi % NX4]
            x4f = x4[:, :]

            if pi + PREFETCH < NPART:
                issue_chunk_inputs(ORDER[pi + PREFETCH],
                                   (pi + PREFETCH) % NX4)

            all_groups = _tile_groups(width)
            if pi >= NPART - 2:
                # tail chunks: tile-major so postproc drains immediately
                halves = [[g] for g in all_groups]
            elif len(all_groups) >= 6:
                halves = [all_groups[0:4], all_groups[4:8]]
            else:
                halves = [all_groups[:(len(all_groups) + 1) // 2],
                          all_groups[(len(all_groups) + 1) // 2:]]

            for groups in halves:
              if not groups:
                  continue
              psums = []
              for (b0, k) in groups:
                ps_t = psumpool.tile([128, k * width], F32, tag="ps")
                psums.append(ps_t)

              # weight-major: each stationary weight is reused across all
              # groups back-to-back so walrus's ldw-opt elides the reloads
              NW = 2 * KSIZE
              for wi in range(NW):
                m, kw = divmod(wi, KSIZE)
                lhsT = (wt1 if m == 0 else wt2)[:, kw, :]
                for gi, (b0, k) in enumerate(groups):
                    s = 2 + (b0 + m) * PW + (kw - 2)
                    rhs = x4f[:, s:s + k * PW].rearrange(
                        "q (b x) -> q b x", x=PW)[:, :, 0:width]
                    pview = psums[gi][:, :].rearrange(
                        "q (b x) -> q b x", x=width)
                    mm = nc.tensor.matmul(
                        pview,
                        lhsT,
                        rhs,
                        start=(wi == 0),
                        stop=(wi == NW - 1),
                    )
                    if prev_mm[0] is not None:
                        bass._add_dep_helper(
                            mm.ins, prev_mm[0].ins, sync=False,
                            reason="pe-stream-order")
                    prev_mm[0] = mm

              # PRELU each group into a shared per-half SBUF tile, then one
              # fused store for the half (DMA descriptors cost ~620ns of
              # queue time each, so fewer+bigger transfers win)
              G = sum(k for (_, k) in groups)
              hb0 = p * NBLK + groups[0][0]
              out_sb = outpool.tile([128, G * width], F32, tag="osb")
              off = 0
              for gi, (b0, k) in enumerate(groups):
                n = k * width
                # single ACT op: out = prelu(psum + bias, alpha)
                # (HW-verified exact; CoreSim lacks Prelu)
                nc.scalar.activation(
                    out_sb[:, off:off + n], psums[gi][:, :],
                    mybir.ActivationFunctionType.Prelu,
                    bias=(bias_t[:, :] if has_bias else 0.0),
                    scale=1.0, alpha=alpha_t[:, :])
                off += n
              # Per-block stores (a fused multi-block store needs a 4-dim
              # AP, which the DMA engine rejects).  All on sync: a store on
              # gpsimd would block later chunks' prefetch loads behind its
              # PRELU dependency (in-order queue).  Near the end gpsimd and
              # scalar have no further work, so spread for faster drain.
              off = 0
              for gi, (b0, k) in enumerate(groups):
                  for bb in range(k):
                      hb = p * NBLK + b0 + bb
                      dst = y_d.ap()[:, 4 * hb:4 * hb + 4, 0:width]
                      if pi >= NPART - 4:
                          # spread stores over sync+scalar (~110GB/s each)
                          # so no backlog remains at the end.  gpsimd never
                          # stores: its software DMA ring has a multi-us
                          # drain-poll latency that would gate kernel end.
                          engs = (nc.sync, nc.scalar)
                      else:
                          engs = (nc.sync,)
                      eng = engs[st_rr[0] % len(engs)]
                      st_rr[0] += 1
                      eng.dma_start(dst, out_sb[:, off:off + width])
                      off += width



    nc.compile()
    return nc


def kernel(x, weight, bias, alpha, widths, _trace=False):
    global LAST_RESULT
    x = np.ascontiguousarray(np.asarray(x, dtype=np.float32))
    weight = np.asarray(weight, dtype=np.float32)
    bias = np.asarray(bias, dtype=np.float32)
    alpha = np.asarray(alpha, dtype=np.float32)
    widths_np = np.asarray(widths, dtype=np.int32)
    wlist = [int(v) for v in widths_np]
    assert x.shape == (B, CI, H, W)
    for wv in wlist:
        # the block-wraparound trick requires masked-zero cols at [510,512)
        assert 4 <= wv <= W - 6, f"width {wv} outside supported range"

    w1, w2 = _build_weights(weight)
    alpha_p = np.ascontiguousarray(np.repeat(alpha, 4)[:, None].astype(np.float32))
    has_bias = bool(np.any(bias != 0.0))

    nc = _build_program(wlist, has_bias)

    shared = {"w1": w1, "w2": w2, "alpha_p": alpha_p}
    if has_bias:
        shared["bias_p"] = np.ascontiguousarray(
            np.repeat(bias, 4)[:, None].astype(np.float32))
    in_maps = [dict(shared, x=np.ascontiguousarray(x[b])) for b in range(B)]

    res = run_bass_kernel_spmd(nc, in_maps, list(range(NCORES)),
                               trace=_trace)
    LAST_RESULT = res
    y = np.stack([res.results[c]["y"] for c in range(NCORES)], axis=0)
    return y.astype(np.float32)


if __name__ == "__main__":
    # smoke test with random data (no reference comparison)
    rng = np.random.default_rng(0)
    x = rng.standard_normal((B, CI, H, W), dtype=np.float32)
    weight = (rng.standard_normal((CO, CI, 5, 5)) * 0.05).astype(np.float32)
    bias = np.zeros(CO, np.float32)
    alpha = np.full(CO, 0.25, np.float32)
    lat = (np.arange(NPART) + 0.5) / NPART * np.pi - np.pi / 2.0
    widths = np.maximum(((np.cos(lat) * W).astype(np.int32) // 2) * 2, 16)
    y = kernel(x, weight, bias, alpha, widths.astype(np.int32))
    print("out", y.shape, y.dtype, float(np.abs(y).max()))

